# revision 9
# baseline (speedup 1.0000x reference)
"""Trainium2 Bass kernel for nn_Attention_10711648436709.

Math (faithful to reference):
    h = einsum('bhik,bhjk->bhij', Q, K) / sqrt(H)     # scale = sqrt(16) = 4
    w = softmax(h, axis=0)                            # over the BATCH axis (B=4)
    out = einsum('bhij,bhjv->bhiv', w, V)
    (mask is a no-op in the reference)

Sharding: head-parallel across 8 cores (16 heads -> 2 heads/core).
Softmax over batch stays core-local => communication-free.

Per-core layout trick: compute transposed scores S^T[j, i] so that
 - QK:  lhsT = K^T[d, j-block]  rhs = Q^T[d, i-chunk]   (host pre-transposes Q,K)
 - PV:  lhsT = V[j-block, v]    rhs = W[j, i-chunk]     (V in natural layout)
 - output accumulates as out^T[v, i] in PSUM; host transposes back.

Batch-0-pivot softmax: g_b = h_b - h_0 (b=1..3) computed by ONE full-K=128
matmul each: lhsT = [K_b^T ; K_0^T], rhs = [Q_b^T ; -Q_0^T] (host packs).
w_b = e^{g_b/4}/(1 + sum e^{g_b'/4}), w_0 = r = 1/(1 + sum ...).

v2 restructure (vs 361us baseline):
 - process jb in PAIRS with a b-major-pair E layout [128, 3*1024] so every
   softmax elementwise op runs once per pair at double FD (amortizes the
   fixed ~58-224 cycle per-op overheads, DVE TT stays in 2x mode on flat
   step-1 views, one 3-plane W multiply per pair).
 - reciprocal alternates per-pair between the ACT (ln(1+x) via bias, then
   exp(-x); both in the natural_log_exp table set) and an optimized DVE
   Newton chain (seed xor+magic fused into ONE dual-op tensor_scalar, and
   the final (2-dx)*x0 sign-fixed with a fused subtract*(-1) so the chain
   yields +r -- the negated-V side input of the old kernel is gone).
"""

import sys
import os

for p in ("/opt/trn_rl_repo",):
    if p not in sys.path:
        sys.path.insert(0, p)

import numpy as np
import ml_dtypes

B, H, S, D = 4, 16, 2048, 64
NCORES = 8
HL = H // NCORES          # 2 heads per core
NB = S // 128             # 16 j-blocks
NP = NB // 2              # 8 j-block pairs
NI = S // 512             # 4 i-chunks

# which jb-pairs (of 8 per (hl,ic)) use the ACT reciprocal path; the rest
# use the DVE Newton path.  Balances ACT vs DVE engine load.
ACT_RECIP_PAIRS = (0, 2, 4, 6)
GPSIMD_T1 = True

TRACE = False
LAST_EXEC_NS = None
LAST_RESULTS = None

_NC = None


def _build_nc():
    import concourse.bass as bass
    import concourse.mybir as mybir
    import concourse.tile as tile

    DT = mybir.dt
    AF = mybir.ActivationFunctionType
    ALU = mybir.AluOpType

    nc = bass.Bass()
    qt = nc.declare_dram_parameter("qt", [3, HL, 128, S], DT.bfloat16, isOutput=False)
    kt = nc.declare_dram_parameter("kt", [3, HL, 128, S], DT.bfloat16, isOutput=False)
    vv = nc.declare_dram_parameter("v", [B, HL, S, D], DT.bfloat16, isOutput=False)
    out = nc.declare_dram_parameter("out", [B, HL, D, S], DT.float32, isOutput=True)

    with tile.TileContext(nc) as tc:
        with (
            tc.tile_pool(name="inputs", bufs=1) as ipool,
            tc.tile_pool(name="work", bufs=3) as wpool,
            tc.tile_pool(name="outsb", bufs=4) as opool,
            tc.tile_pool(name="qkps", bufs=2, space="PSUM") as qkpool,
            tc.tile_pool(name="ops", bufs=1, space="PSUM") as opsum,
        ):
            QT = ipool.tile([128, 3 * HL * S], DT.bfloat16, tag="qt")
            KT = ipool.tile([128, 3 * HL * S], DT.bfloat16, tag="kt")
            VA = ipool.tile([128, B * HL * NB * D], DT.bfloat16, tag="va")
            # load order = consumption order: hl=0's K/Q first (first QK MMs),
            # then its V (first PV is 2 pipeline steps in), then hl=1.
            for hl in range(HL):
                for bb in range(3):
                    off = (bb * HL + hl) * S
                    nc.sync.dma_start(out=KT[:, off : off + S], in_=kt[bb, hl])
                    nc.sync.dma_start(out=QT[:, off : off + S], in_=qt[bb, hl])
                for b in range(B):
                    voff = (b * HL + hl) * NB * D
                    nc.sync.dma_start(
                        out=VA[:, voff : voff + NB * D].rearrange(
                            "p (n d) -> p n d", d=D
                        ),
                        in_=vv[b, hl].rearrange("(n p) d -> p n d", p=128),
                    )

            # 2-stage software pipeline across jb-pairs, flattened over the
            # whole (hl, ic) space: at step t we emit QK+exp for pair t,
            # softmax (DVE/ACT) for pair t-1, and PV matmuls for pair t-2.
            # Every instruction entering an engine FIFO then has its cross-
            # engine deps a full pair-stage old -> no head-of-line stalls,
            # and PE never idles long enough to re-throttle HAM.
            NPAIRS = HL * NI * NP
            pos = {}        # (hl, ic) -> po tiles, created lazily
            state = {}      # pair index -> dict(E, rb, W, jp, hl, ic)

            def pair_coords(t):
                hl, r = divmod(t, NI * NP)
                ic, jp = divmod(r, NP)
                return hl, ic, jp

            def emit_qk_exp(t):
                hl, ic, jp = pair_coords(t)
                E = wpool.tile([128, 3072], DT.bfloat16, tag="E")
                E3d = E.rearrange("p (b n) -> p b n", b=3)
                for h in range(2):
                    jb = 2 * jp + h
                    qk = qkpool.tile([128, 1536], DT.float32, tag="qk")
                    for bb in range(3):
                        off = (bb * HL + hl) * S
                        nc.tensor.matmul(
                            qk[:, bb * 512 : (bb + 1) * 512],
                            lhsT=KT[:, off + jb * 128 : off + jb * 128 + 128],
                            rhs=QT[:, off + ic * 512 : off + ic * 512 + 512],
                            start=True,
                            stop=True,
                        )
                    nc.scalar.activation(
                        E3d[:, :, h * 512 : (h + 1) * 512],
                        qk.rearrange("p (b n) -> p b n", b=3),
                        AF.Exp,
                        scale=0.25,
                    )
                state[t] = {"E": E, "hl": hl, "ic": ic, "jp": jp}

            def emit_softmax(t):
                st = state[t]
                E = st["E"]
                E3d = E.rearrange("p (b n) -> p b n", b=3)
                jp = st["jp"]
                # T1 = E1 + E2, T2 = T1 + E3  (flat [128,1024] bf16, 2x)
                T1 = wpool.tile([128, 1024], DT.bfloat16, tag="T1")
                if GPSIMD_T1:
                    nc.gpsimd.tensor_add(T1, E[:, 0:1024], E[:, 1024:2048])
                else:
                    nc.vector.tensor_add(T1, E[:, 0:1024], E[:, 1024:2048])
                T2 = wpool.tile([128, 1024], DT.bfloat16, tag="T2")
                nc.vector.tensor_add(T2, T1, E[:, 2048:3072])
                rb = wpool.tile([128, 1024], DT.bfloat16, tag="rb")
                if jp in ACT_RECIP_PAIRS:
                    # r = exp(-ln(1 + T2)); Ln and Exp share the
                    # natural_log_exp table set; +1 rides the bias.
                    lnd = wpool.tile([128, 1024], DT.float32, tag="lnd")
                    nc.scalar.activation(lnd, T2, AF.Ln, bias=1.0)
                    nc.scalar.activation(rb, lnd, AF.Exp, scale=-1.0)
                else:
                    # DVE Newton: den = T2 + 1; x0 = bf16 bit seed;
                    # rb = (2 - den*x0) * x0  (sign-fixed to +r).
                    den = wpool.tile([128, 1024], DT.bfloat16, tag="den")
                    nc.vector.tensor_scalar(
                        out=den,
                        in0=T2,
                        scalar1=1.0,
                        scalar2=None,
                        op0=ALU.add,
                    )
                    x0 = wpool.tile([128, 1024], DT.bfloat16, tag="x0")
                    # seed bits = 0x7EF4 + ~den = 0x7EF3 - den, as
                    # mult(-1)+add so both TS ops are arith (the
                    # birverifier rejects mixed bitwise/arith pairs);
                    # int16 values are exact in the fp32 datapath.
                    nc.vector.tensor_scalar(
                        out=x0.bitcast(DT.int16),
                        in0=den.bitcast(DT.int16),
                        scalar1=-1,
                        scalar2=0x7EF3,
                        op0=ALU.mult,
                        op1=ALU.add,
                    )
                    dx = wpool.tile([128, 1024], DT.bfloat16, tag="dx")
                    nc.vector.tensor_mul(dx, den, x0)
                    tp = wpool.tile([128, 1024], DT.bfloat16, tag="tp")
                    # tp = (dx - 2) * (-1) = 2 - dx in one dual-op TS
                    nc.vector.tensor_scalar(
                        out=tp,
                        in0=dx,
                        scalar1=2.0,
                        scalar2=-1.0,
                        op0=ALU.subtract,
                        op1=ALU.mult,
                    )
                    nc.vector.tensor_mul(rb, tp, x0)
                # W_b = E_b * r for b=1..3, one 3-plane broadcast mul
                W = wpool.tile([128, 3072], DT.bfloat16, tag="W")
                nc.vector.tensor_mul(
                    W.rearrange("p (b n) -> p b n", b=3),
                    E3d,
                    rb.unsqueeze(1).broadcast_to([128, 3, 1024]),
                )
                st["rb"] = rb
                st["W"] = W

            def emit_pv(t):
                st = state.pop(t)
                hl, ic, jp = st["hl"], st["ic"], st["jp"]
                rb, W = st["rb"], st["W"]
                if jp == 0:
                    pos[(hl, ic)] = [
                        opsum.tile(
                            [128, 512], DT.float32, tag=f"po{p}", name=f"po{p}"
                        )
                        for p in range(2)
                    ]
                po = pos[(hl, ic)]
                for h in range(2):
                    jb = 2 * jp + h
                    rhss = [
                        rb[:, h * 512 : (h + 1) * 512],
                        W[:, h * 512 : h * 512 + 512],
                        W[:, 1024 + h * 512 : 1024 + h * 512 + 512],
                        W[:, 2048 + h * 512 : 2048 + h * 512 + 512],
                    ]
                    for p in range(2):
                        for half in range(2):
                            b = 2 * p + half
                            voff = (b * HL + hl) * NB * D + jb * D
                            nc.tensor.matmul(
                                po[p][64 * half : 64 * (half + 1), :],
                                lhsT=VA[:, voff : voff + D],
                                rhs=rhss[b],
                                start=(jb == 0),
                                stop=(jb == NB - 1),
                                tile_position=(0, 64 * half),
                            )
                if jp == NP - 1:
                    po = pos.pop((hl, ic))
                    for p in range(2):
                        for half in range(2):
                            b = 2 * p + half
                            osb = opool.tile([D, 512], DT.float32, tag="osb")
                            src = po[p][64 * half : 64 * (half + 1), :]
                            # split the PSUM->SBUF drains between ACT and
                            # DVE to balance engine load
                            if half == 0:
                                nc.scalar.copy(osb, src)
                            else:
                                nc.vector.tensor_copy(osb, src)
                            nc.sync.dma_start(
                                out=out[b, hl, :, ic * 512 : (ic + 1) * 512],
                                in_=osb,
                            )

            # PV (oldest deps) first within each step so PSUM drains and PV
            # matmuls queue ahead of the step's fresh work on each engine.
            for t in range(NPAIRS + 2):
                if t >= 2:
                    emit_pv(t - 2)
                if t < NPAIRS:
                    emit_qk_exp(t)
                if 1 <= t <= NPAIRS:
                    emit_softmax(t - 1)
    return nc


def _patch_bir_waits(bir_json: bytes) -> bytes:
    """This walrus build only accepts 1 sync wait per instruction (2 for
    DMACopy); Tile emits more. Legalize:
      1. merge duplicate-semaphore waits (keep max threshold),
      2. drop waits that are transitively implied (vector-clock replay over
         the straight-line program: in-order completion per engine, FIFO per
         DMA queue, and the knowledge a producer had when it bumped a sem),
      3. split any residual multi-wait onto injected EventSemaphore
         instructions on the same engine right before the instruction.
    Only monotonic sem-inc/sem-ge-imm semaphores participate in (2); barrier
    sems (dec/eq) are left untouched."""
    import json
    from collections import defaultdict

    bir = json.loads(bir_json)

    for fn in bir["functions"]:
        insts = []
        for bb in fn["blocks"]:
            for inst in bb.get("instructions", []):
                insts.append(inst)

        # classify sems: monotonic = all updates are positive sem-inc and
        # all waits are sem-ge-imm
        bad_sems = set()
        for inst in insts:
            si = inst.get("sync_info") or {}
            for u in si.get("on_update") or []:
                if u.get("update_mode") != "sem-inc" or u.get("update_value", 0) <= 0:
                    bad_sems.add(u["id"])
            for w in si.get("on_wait") or []:
                if w.get("wait_mode") != "sem-ge-imm":
                    bad_sems.add(w["id"])

        # proc of an instruction: its engine stream, except DMACopy whose
        # completion (and sem update) is FIFO per DMA queue, keyed by the
        # sem it updates.
        def proc_of(inst):
            if inst.get("opcode") == "DMACopy":
                si = inst.get("sync_info") or {}
                ups = si.get("on_update") or []
                if ups:
                    return ("dma", ups[0]["id"])
            return ("eng", inst.get("engine"))

        sem_val = defaultdict(int)          # current cumulative value per sem
        producers = defaultdict(list)       # sem -> [(value_after, CK dict)]
        know = defaultdict(dict)            # proc -> {sem: guaranteed min}

        def join(dst, src):
            for s, v in src.items():
                if dst.get(s, 0) < v:
                    dst[s] = v

        out_blocks = {id(bb): [] for bb in fn["blocks"]}
        inj = 0
        for bb in fn["blocks"]:
            new_list = []
            for inst in bb.get("instructions", []):
                p = proc_of(inst)
                eng_p = ("eng", inst.get("engine"))
                # waits on a DMACopy are enforced by the DGE queue (FIFO per
                # queue), not the issuing engine — track knowledge per queue
                kp = p if p[0] == "dma" else eng_p
                si = inst.get("sync_info") or {}
                waits = si.get("on_wait") or []
                # merge duplicate sems
                merged = {}
                for w in waits:
                    k = w["id"]
                    if k not in merged or w.get("wait_value", 0) > merged[k].get(
                        "wait_value", 0
                    ):
                        merged[k] = w
                waits = list(merged.values())
                kept = []
                for w in waits:
                    s, v = w["id"], w.get("wait_value", 0)
                    if s in bad_sems:
                        kept.append(w)
                        continue
                    if know[kp].get(s, 0) >= v:
                        continue  # redundant
                    kept.append(w)
                    know[kp][s] = max(know[kp].get(s, 0), v)
                    # transitive knowledge from the producer that reached v
                    for val_after, ck in producers[s]:
                        if val_after >= v:
                            join(know[kp], ck)
                            break
                # split if too many waits remain
                budget = 1
                while len(kept) > budget:
                    w = kept.pop(0)
                    inj += 1
                    new_list.append(
                        {
                            "debug": inst.get("debug", 0),
                            "engine": inst.get("engine"),
                            "ins": [],
                            "name": f"WS-{inj}-{inst.get('name')}",
                            "opcode": "EventSemaphore",
                            "outs": [],
                            "sync_info": {"on_update": [], "on_wait": [w]},
                        }
                    )
                si["on_wait"] = kept
                inst["sync_info"] = si
                new_list.append(inst)
                # apply this instruction's updates for downstream knowledge
                ups = si.get("on_update") or []
                ck = None
                for u in ups:
                    s = u["id"]
                    if s in bad_sems:
                        continue
                    sem_val[s] += u.get("update_value", 0)
                    if ck is None:
                        # completion knowledge: what this proc knew here
                        # (for DMA: queue knowledge + engine state at issue)
                        ck = dict(know[kp])
                        if p[0] == "dma":
                            join(ck, know[eng_p])
                    ck[s] = sem_val[s]
                    producers[s].append((sem_val[s], ck))
                # a proc knows its own sems' values after completion
                if p[0] == "eng":
                    for u in ups:
                        if u["id"] not in bad_sems:
                            know[eng_p][u["id"]] = sem_val[u["id"]]
            out_blocks[id(bb)] = new_list
        for bb in fn["blocks"]:
            bb["instructions"] = out_blocks[id(bb)]
    return json.dumps(bir).encode()


_PATCHED = False


def _install_bir_patch():
    global _PATCHED
    if _PATCHED:
        return
    import concourse.bass2jax as bass2jax
    from concourse import bass_utils as _bu

    orig = _bu.compile_bir_kernel

    def patched(bir_json, tmpdir, neff_name="file.neff"):
        return orig(_patch_bir_waits(bir_json), tmpdir, neff_name)

    bass2jax.compile_bir_kernel = patched
    # keep profile artifacts local — no bucket in this environment
    _bu.upload_artifacts = lambda tmpdir: str(tmpdir)
    _PATCHED = True


def _install_ntff_shim():
    """run_bass_kernel_spmd(trace=True) under axon needs
    antenv.axon_hooks.get_axon_ntff_profile_hook; the module isn't staged in
    this image, but libaxon_pjrt.so exposes the profile C ABI — recreate the
    shim (same recipe as trn_agent_boot)."""
    import sys as _sys

    if "antenv.axon_hooks" in _sys.modules:
        return
    import contextlib
    import ctypes
    import types

    import antenv  # noqa: F401

    so_path = "/opt/axon/libaxon_pjrt.so"
    hook = None
    try:
        lib = ctypes.CDLL(so_path)
        if hasattr(lib, "axon_start_nrt_profile"):
            lib.axon_start_nrt_profile.argtypes = [
                ctypes.POINTER(ctypes.c_int64),
                ctypes.c_size_t,
            ]
            lib.axon_start_nrt_profile.restype = ctypes.c_int64
            lib.axon_stop_nrt_profile.argtypes = [ctypes.c_char_p]
            lib.axon_stop_nrt_profile.restype = ctypes.c_int64

            @contextlib.contextmanager
            def hook(output_dir, device_ids):
                import jax

                jax.devices()
                if device_ids:
                    ids = (ctypes.c_int64 * len(device_ids))(*device_ids)
                    rc = lib.axon_start_nrt_profile(ids, len(device_ids))
                else:
                    rc = lib.axon_start_nrt_profile(None, 0)
                if rc != 0:
                    raise RuntimeError(f"axon_start_nrt_profile rc={rc}")
                try:
                    yield
                finally:
                    n = lib.axon_stop_nrt_profile(str(output_dir).encode())
                    print(
                        f"ntff profile: {n} file(s) -> {output_dir}",
                        file=_sys.stderr,
                    )
    except OSError:
        pass

    mod = types.ModuleType("antenv.axon_hooks")
    mod.get_axon_ntff_profile_hook = lambda: hook
    mod.set_axon_ntff_profile_hook = lambda h: None
    _sys.modules["antenv.axon_hooks"] = mod
    import antenv as _ae

    _ae.axon_hooks = mod


def kernel(query, key, value, mask=None):
    global _NC, LAST_EXEC_NS, LAST_RESULTS
    from concourse.bass_utils import run_bass_kernel_spmd

    _install_bir_patch()
    if TRACE:
        _install_ntff_shim()

    query = np.asarray(query, dtype=np.float32)
    key = np.asarray(key, dtype=np.float32)
    value = np.asarray(value, dtype=np.float32)

    if _NC is None:
        _NC = _build_nc()
    nc = _NC

    bf16 = ml_dtypes.bfloat16

    def pack_pivot(x, negate_base):
        # [B, HL, S, D] -> [B, HL, D, S]; stack [x_b^T ; (+-)x_0^T] on the
        # partition axis for b = 1..3 -> [3, HL, 128, S]
        xt = x.transpose(0, 1, 3, 2)  # [B, HL, D, S]
        base = -xt[0] if negate_base else xt[0]  # [HL, D, S]
        stk = np.stack(
            [np.concatenate([xt[b], base], axis=1) for b in (1, 2, 3)], axis=0
        )
        return np.ascontiguousarray(stk).astype(bf16)

    in_maps = []
    for c in range(NCORES):
        hs = slice(HL * c, HL * (c + 1))
        qt = pack_pivot(query[:, hs], negate_base=True)
        kt = pack_pivot(key[:, hs], negate_base=False)
        vc = np.ascontiguousarray(value[:, hs]).astype(bf16)
        in_maps.append({"qt": qt, "kt": kt, "v": vc})

    res = run_bass_kernel_spmd(
        nc, in_maps, core_ids=list(range(NCORES)), trace=TRACE
    )
    LAST_RESULTS = res
    LAST_EXEC_NS = getattr(res, "exec_time_ns", None)

    full = np.empty((B, H, S, D), dtype=np.float32)
    for c in range(NCORES):
        o = np.asarray(res.results[c]["out"])  # [B, HL, D, S]
        full[:, HL * c : HL * (c + 1)] = o.transpose(0, 1, 3, 2)
    return full


# revision 10
# speedup vs baseline: 1.2379x; 1.2379x over previous
"""Trainium2 Bass kernel for nn_Attention_10711648436709.

Math (faithful to reference):
    h = einsum('bhik,bhjk->bhij', Q, K) / sqrt(H)     # scale = sqrt(16) = 4
    w = softmax(h, axis=0)                            # over the BATCH axis (B=4)
    out = einsum('bhij,bhjv->bhiv', w, V)
    (mask is a no-op in the reference)

Sharding: head-parallel across 8 cores (16 heads -> 2 heads/core).
Softmax over batch stays core-local => communication-free.

Per-core layout trick: compute transposed scores S^T[j, i] so that
 - QK:  lhsT = K^T[d, j-block]  rhs = Q^T[d, i-chunk]   (host pre-transposes Q,K)
 - PV:  lhsT = V[j-block, v]    rhs = W[j, i-chunk]     (V in natural layout)
 - output accumulates as out^T[v, i] in PSUM; host transposes back.

Batch-0-pivot softmax: g_b = h_b - h_0 (b=1..3) computed by ONE full-K=128
matmul each: lhsT = [K_b^T ; K_0^T], rhs = [Q_b^T ; -Q_0^T] (host packs).
w_b = e^{g_b/4}/(1 + sum e^{g_b'/4}), w_0 = r = 1/(1 + sum ...).

v2 restructure (vs 361us baseline):
 - process jb in PAIRS with a b-major-pair E layout [128, 3*1024] so every
   softmax elementwise op runs once per pair at double FD (amortizes the
   fixed ~58-224 cycle per-op overheads, DVE TT stays in 2x mode on flat
   step-1 views, one 3-plane W multiply per pair).
 - reciprocal alternates per-pair between the ACT (ln(1+x) via bias, then
   exp(-x); both in the natural_log_exp table set) and an optimized DVE
   Newton chain (seed xor+magic fused into ONE dual-op tensor_scalar, and
   the final (2-dx)*x0 sign-fixed with a fused subtract*(-1) so the chain
   yields +r -- the negated-V side input of the old kernel is gone).
"""

import sys
import os

for p in ("/opt/trn_rl_repo",):
    if p not in sys.path:
        sys.path.insert(0, p)

import numpy as np
import ml_dtypes

B, H, S, D = 4, 16, 2048, 64
NCORES = 8
HL = H // NCORES          # 2 heads per core
NB = S // 128             # 16 j-blocks
NP = NB // 2              # 8 j-block pairs
NI = S // 512             # 4 i-chunks

# which jb-pairs (of 8 per (hl,ic)) use the ACT reciprocal path; the rest
# use the DVE Newton path.  Balances ACT vs DVE engine load.
ACT_RECIP_PAIRS = (0, 2, 4, 6)
# GPSIMD elementwise offload measured: TT [128,1024] = 2.6us AND its shared
# SBUF port inflates DVE/ACT/PE op times ~20% across the board. Never use.
GPSIMD_T1 = False

TRACE = False
LAST_EXEC_NS = None
LAST_RESULTS = None

_NC = None


def _build_nc():
    import concourse.bass as bass
    import concourse.mybir as mybir
    import concourse.tile as tile

    DT = mybir.dt
    AF = mybir.ActivationFunctionType
    ALU = mybir.AluOpType

    nc = bass.Bass()
    qt = nc.declare_dram_parameter("qt", [3, HL, 128, S], DT.bfloat16, isOutput=False)
    kt = nc.declare_dram_parameter("kt", [3, HL, 128, S], DT.bfloat16, isOutput=False)
    vv = nc.declare_dram_parameter("v", [B, HL, S, D], DT.bfloat16, isOutput=False)
    out = nc.declare_dram_parameter("out", [B, HL, D, S], DT.float32, isOutput=True)

    with tile.TileContext(nc) as tc:
        with (
            tc.tile_pool(name="inputs", bufs=1) as ipool,
            tc.tile_pool(name="work", bufs=3) as wpool,
            tc.tile_pool(name="outsb", bufs=4) as opool,
            tc.tile_pool(name="qkps", bufs=2, space="PSUM") as qkpool,
            tc.tile_pool(name="ops", bufs=1, space="PSUM") as opsum,
        ):
            QT = ipool.tile([128, 3 * HL * S], DT.bfloat16, tag="qt")
            KT = ipool.tile([128, 3 * HL * S], DT.bfloat16, tag="kt")
            VA = ipool.tile([128, B * HL * NB * D], DT.bfloat16, tag="va")
            # load order = consumption order: hl=0's K/Q first (first QK MMs),
            # then its V (first PV is 2 pipeline steps in), then hl=1.
            for hl in range(HL):
                for bb in range(3):
                    off = (bb * HL + hl) * S
                    nc.sync.dma_start(out=KT[:, off : off + S], in_=kt[bb, hl])
                    nc.sync.dma_start(out=QT[:, off : off + S], in_=qt[bb, hl])
                for b in range(B):
                    voff = (b * HL + hl) * NB * D
                    nc.sync.dma_start(
                        out=VA[:, voff : voff + NB * D].rearrange(
                            "p (n d) -> p n d", d=D
                        ),
                        in_=vv[b, hl].rearrange("(n p) d -> p n d", p=128),
                    )

            # 2-stage software pipeline across jb-pairs, flattened over the
            # whole (hl, ic) space: at step t we emit QK+exp for pair t,
            # softmax (DVE/ACT) for pair t-1, and PV matmuls for pair t-2.
            # Every instruction entering an engine FIFO then has its cross-
            # engine deps a full pair-stage old -> no head-of-line stalls,
            # and PE never idles long enough to re-throttle HAM.
            NPAIRS = HL * NI * NP
            pos = {}        # (hl, ic) -> po tiles, created lazily
            state = {}      # pair index -> dict(E, rb, W, jp, hl, ic)

            def pair_coords(t):
                hl, r = divmod(t, NI * NP)
                ic, jp = divmod(r, NP)
                return hl, ic, jp

            def emit_qk_exp(t):
                hl, ic, jp = pair_coords(t)
                E = wpool.tile([128, 3072], DT.bfloat16, tag="E")
                E3d = E.rearrange("p (b n) -> p b n", b=3)
                for h in range(2):
                    jb = 2 * jp + h
                    qk = qkpool.tile([128, 1536], DT.float32, tag="qk")
                    for bb in range(3):
                        off = (bb * HL + hl) * S
                        nc.tensor.matmul(
                            qk[:, bb * 512 : (bb + 1) * 512],
                            lhsT=KT[:, off + jb * 128 : off + jb * 128 + 128],
                            rhs=QT[:, off + ic * 512 : off + ic * 512 + 512],
                            start=True,
                            stop=True,
                        )
                    nc.scalar.activation(
                        E3d[:, :, h * 512 : (h + 1) * 512],
                        qk.rearrange("p (b n) -> p b n", b=3),
                        AF.Exp,
                        scale=0.25,
                    )
                state[t] = {"E": E, "hl": hl, "ic": ic, "jp": jp}

            def emit_softmax(t):
                st = state[t]
                E = st["E"]
                E3d = E.rearrange("p (b n) -> p b n", b=3)
                jp = st["jp"]
                # T1 = E1 + E2, T2 = T1 + E3  (flat [128,1024] bf16, 2x)
                T1 = wpool.tile([128, 1024], DT.bfloat16, tag="T1")
                if GPSIMD_T1:
                    nc.gpsimd.tensor_add(T1, E[:, 0:1024], E[:, 1024:2048])
                else:
                    nc.vector.tensor_add(T1, E[:, 0:1024], E[:, 1024:2048])
                T2 = wpool.tile([128, 1024], DT.bfloat16, tag="T2")
                nc.vector.tensor_add(T2, T1, E[:, 2048:3072])
                rb = wpool.tile([128, 1024], DT.bfloat16, tag="rb")
                if jp in ACT_RECIP_PAIRS:
                    # r = exp(-ln(1 + T2)); Ln and Exp share the
                    # natural_log_exp table set; +1 rides the bias.
                    lnd = wpool.tile([128, 1024], DT.float32, tag="lnd")
                    nc.scalar.activation(lnd, T2, AF.Ln, bias=1.0)
                    nc.scalar.activation(rb, lnd, AF.Exp, scale=-1.0)
                else:
                    # DVE Newton: den = T2 + 1; x0 = bf16 bit seed;
                    # rb = (2 - den*x0) * x0  (sign-fixed to +r).
                    den = wpool.tile([128, 1024], DT.bfloat16, tag="den")
                    nc.vector.tensor_scalar(
                        out=den,
                        in0=T2,
                        scalar1=1.0,
                        scalar2=None,
                        op0=ALU.add,
                    )
                    x0 = wpool.tile([128, 1024], DT.bfloat16, tag="x0")
                    # seed bits = 0x7EF4 + ~den = 0x7EF3 - den, as
                    # mult(-1)+add so both TS ops are arith (the
                    # birverifier rejects mixed bitwise/arith pairs);
                    # int16 values are exact in the fp32 datapath.
                    nc.vector.tensor_scalar(
                        out=x0.bitcast(DT.int16),
                        in0=den.bitcast(DT.int16),
                        scalar1=-1,
                        scalar2=0x7EF3,
                        op0=ALU.mult,
                        op1=ALU.add,
                    )
                    dx = wpool.tile([128, 1024], DT.bfloat16, tag="dx")
                    nc.vector.tensor_mul(dx, den, x0)
                    tp = wpool.tile([128, 1024], DT.bfloat16, tag="tp")
                    # tp = (dx - 2) * (-1) = 2 - dx in one dual-op TS
                    nc.vector.tensor_scalar(
                        out=tp,
                        in0=dx,
                        scalar1=2.0,
                        scalar2=-1.0,
                        op0=ALU.subtract,
                        op1=ALU.mult,
                    )
                    nc.vector.tensor_mul(rb, tp, x0)
                # W_b = E_b * r for b=1..3, one 3-plane broadcast mul
                W = wpool.tile([128, 3072], DT.bfloat16, tag="W")
                nc.vector.tensor_mul(
                    W.rearrange("p (b n) -> p b n", b=3),
                    E3d,
                    rb.unsqueeze(1).broadcast_to([128, 3, 1024]),
                )
                st["rb"] = rb
                st["W"] = W

            def emit_pv(t):
                st = state.pop(t)
                hl, ic, jp = st["hl"], st["ic"], st["jp"]
                rb, W = st["rb"], st["W"]
                if jp == 0:
                    pos[(hl, ic)] = [
                        opsum.tile(
                            [128, 512], DT.float32, tag=f"po{p}", name=f"po{p}"
                        )
                        for p in range(2)
                    ]
                po = pos[(hl, ic)]
                for h in range(2):
                    jb = 2 * jp + h
                    rhss = [
                        rb[:, h * 512 : (h + 1) * 512],
                        W[:, h * 512 : h * 512 + 512],
                        W[:, 1024 + h * 512 : 1024 + h * 512 + 512],
                        W[:, 2048 + h * 512 : 2048 + h * 512 + 512],
                    ]
                    for p in range(2):
                        for half in range(2):
                            b = 2 * p + half
                            voff = (b * HL + hl) * NB * D + jb * D
                            nc.tensor.matmul(
                                po[p][64 * half : 64 * (half + 1), :],
                                lhsT=VA[:, voff : voff + D],
                                rhs=rhss[b],
                                start=(jb == 0),
                                stop=(jb == NB - 1),
                                tile_position=(0, 64 * half),
                            )
                if jp == NP - 1:
                    po = pos.pop((hl, ic))
                    for p in range(2):
                        for half in range(2):
                            b = 2 * p + half
                            osb = opool.tile([D, 512], DT.float32, tag="osb")
                            src = po[p][64 * half : 64 * (half + 1), :]
                            # split the PSUM->SBUF drains between ACT and
                            # DVE to balance engine load
                            if half == 0:
                                nc.scalar.copy(osb, src)
                            else:
                                nc.vector.tensor_copy(osb, src)
                            nc.sync.dma_start(
                                out=out[b, hl, :, ic * 512 : (ic + 1) * 512],
                                in_=osb,
                            )

            # PV (oldest deps) first within each step so PSUM drains and PV
            # matmuls queue ahead of the step's fresh work on each engine.
            for t in range(NPAIRS + 2):
                if t >= 2:
                    emit_pv(t - 2)
                if t < NPAIRS:
                    emit_qk_exp(t)
                if 1 <= t <= NPAIRS:
                    emit_softmax(t - 1)
    return nc


def _patch_bir_waits(bir_json: bytes) -> bytes:
    """This walrus build only accepts 1 sync wait per instruction (2 for
    DMACopy); Tile emits more. Legalize:
      1. merge duplicate-semaphore waits (keep max threshold),
      2. drop waits that are transitively implied (vector-clock replay over
         the straight-line program: in-order completion per engine, FIFO per
         DMA queue, and the knowledge a producer had when it bumped a sem),
      3. split any residual multi-wait onto injected EventSemaphore
         instructions on the same engine right before the instruction.
    Only monotonic sem-inc/sem-ge-imm semaphores participate in (2); barrier
    sems (dec/eq) are left untouched."""
    import json
    from collections import defaultdict

    bir = json.loads(bir_json)

    for fn in bir["functions"]:
        insts = []
        for bb in fn["blocks"]:
            for inst in bb.get("instructions", []):
                insts.append(inst)

        # classify sems: monotonic = all updates are positive sem-inc and
        # all waits are sem-ge-imm
        bad_sems = set()
        for inst in insts:
            si = inst.get("sync_info") or {}
            for u in si.get("on_update") or []:
                if u.get("update_mode") != "sem-inc" or u.get("update_value", 0) <= 0:
                    bad_sems.add(u["id"])
            for w in si.get("on_wait") or []:
                if w.get("wait_mode") != "sem-ge-imm":
                    bad_sems.add(w["id"])

        # proc of an instruction: its engine stream, except DMACopy whose
        # completion (and sem update) is FIFO per DMA queue, keyed by the
        # sem it updates.
        def proc_of(inst):
            if inst.get("opcode") == "DMACopy":
                si = inst.get("sync_info") or {}
                ups = si.get("on_update") or []
                if ups:
                    return ("dma", ups[0]["id"])
            return ("eng", inst.get("engine"))

        sem_val = defaultdict(int)          # current cumulative value per sem
        producers = defaultdict(list)       # sem -> [(value_after, CK dict)]
        know = defaultdict(dict)            # proc -> {sem: guaranteed min}

        def join(dst, src):
            for s, v in src.items():
                if dst.get(s, 0) < v:
                    dst[s] = v

        out_blocks = {id(bb): [] for bb in fn["blocks"]}
        inj = 0
        for bb in fn["blocks"]:
            new_list = []
            for inst in bb.get("instructions", []):
                p = proc_of(inst)
                eng_p = ("eng", inst.get("engine"))
                # waits on a DMACopy are enforced by the DGE queue (FIFO per
                # queue), not the issuing engine — track knowledge per queue
                kp = p if p[0] == "dma" else eng_p
                si = inst.get("sync_info") or {}
                waits = si.get("on_wait") or []
                # merge duplicate sems
                merged = {}
                for w in waits:
                    k = w["id"]
                    if k not in merged or w.get("wait_value", 0) > merged[k].get(
                        "wait_value", 0
                    ):
                        merged[k] = w
                waits = list(merged.values())
                kept = []
                for w in waits:
                    s, v = w["id"], w.get("wait_value", 0)
                    if s in bad_sems:
                        kept.append(w)
                        continue
                    if know[kp].get(s, 0) >= v:
                        continue  # redundant
                    kept.append(w)
                    know[kp][s] = max(know[kp].get(s, 0), v)
                    # transitive knowledge from the producer that reached v
                    for val_after, ck in producers[s]:
                        if val_after >= v:
                            join(know[kp], ck)
                            break
                # split if too many waits remain
                budget = 1
                while len(kept) > budget:
                    w = kept.pop(0)
                    inj += 1
                    new_list.append(
                        {
                            "debug": inst.get("debug", 0),
                            "engine": inst.get("engine"),
                            "ins": [],
                            "name": f"WS-{inj}-{inst.get('name')}",
                            "opcode": "EventSemaphore",
                            "outs": [],
                            "sync_info": {"on_update": [], "on_wait": [w]},
                        }
                    )
                si["on_wait"] = kept
                inst["sync_info"] = si
                new_list.append(inst)
                # apply this instruction's updates for downstream knowledge
                ups = si.get("on_update") or []
                ck = None
                for u in ups:
                    s = u["id"]
                    if s in bad_sems:
                        continue
                    sem_val[s] += u.get("update_value", 0)
                    if ck is None:
                        # completion knowledge: what this proc knew here
                        # (for DMA: queue knowledge + engine state at issue)
                        ck = dict(know[kp])
                        if p[0] == "dma":
                            join(ck, know[eng_p])
                    ck[s] = sem_val[s]
                    producers[s].append((sem_val[s], ck))
                # a proc knows its own sems' values after completion
                if p[0] == "eng":
                    for u in ups:
                        if u["id"] not in bad_sems:
                            know[eng_p][u["id"]] = sem_val[u["id"]]
            out_blocks[id(bb)] = new_list
        for bb in fn["blocks"]:
            bb["instructions"] = out_blocks[id(bb)]
    return json.dumps(bir).encode()


_PATCHED = False


def _install_bir_patch():
    global _PATCHED
    if _PATCHED:
        return
    import concourse.bass2jax as bass2jax
    from concourse import bass_utils as _bu

    orig = _bu.compile_bir_kernel

    def patched(bir_json, tmpdir, neff_name="file.neff"):
        return orig(_patch_bir_waits(bir_json), tmpdir, neff_name)

    bass2jax.compile_bir_kernel = patched
    # keep profile artifacts local — no bucket in this environment
    _bu.upload_artifacts = lambda tmpdir: str(tmpdir)
    _PATCHED = True


def _install_ntff_shim():
    """run_bass_kernel_spmd(trace=True) under axon needs
    antenv.axon_hooks.get_axon_ntff_profile_hook; the module isn't staged in
    this image, but libaxon_pjrt.so exposes the profile C ABI — recreate the
    shim (same recipe as trn_agent_boot)."""
    import sys as _sys

    if "antenv.axon_hooks" in _sys.modules:
        return
    import contextlib
    import ctypes
    import types

    import antenv  # noqa: F401

    so_path = "/opt/axon/libaxon_pjrt.so"
    hook = None
    try:
        lib = ctypes.CDLL(so_path)
        if hasattr(lib, "axon_start_nrt_profile"):
            lib.axon_start_nrt_profile.argtypes = [
                ctypes.POINTER(ctypes.c_int64),
                ctypes.c_size_t,
            ]
            lib.axon_start_nrt_profile.restype = ctypes.c_int64
            lib.axon_stop_nrt_profile.argtypes = [ctypes.c_char_p]
            lib.axon_stop_nrt_profile.restype = ctypes.c_int64

            @contextlib.contextmanager
            def hook(output_dir, device_ids):
                import jax

                jax.devices()
                if device_ids:
                    ids = (ctypes.c_int64 * len(device_ids))(*device_ids)
                    rc = lib.axon_start_nrt_profile(ids, len(device_ids))
                else:
                    rc = lib.axon_start_nrt_profile(None, 0)
                if rc != 0:
                    raise RuntimeError(f"axon_start_nrt_profile rc={rc}")
                try:
                    yield
                finally:
                    n = lib.axon_stop_nrt_profile(str(output_dir).encode())
                    print(
                        f"ntff profile: {n} file(s) -> {output_dir}",
                        file=_sys.stderr,
                    )
    except OSError:
        pass

    mod = types.ModuleType("antenv.axon_hooks")
    mod.get_axon_ntff_profile_hook = lambda: hook
    mod.set_axon_ntff_profile_hook = lambda h: None
    _sys.modules["antenv.axon_hooks"] = mod
    import antenv as _ae

    _ae.axon_hooks = mod


def kernel(query, key, value, mask=None):
    global _NC, LAST_EXEC_NS, LAST_RESULTS
    from concourse.bass_utils import run_bass_kernel_spmd

    _install_bir_patch()
    if TRACE:
        _install_ntff_shim()

    query = np.asarray(query, dtype=np.float32)
    key = np.asarray(key, dtype=np.float32)
    value = np.asarray(value, dtype=np.float32)

    if _NC is None:
        _NC = _build_nc()
    nc = _NC

    bf16 = ml_dtypes.bfloat16

    def pack_pivot(x, negate_base):
        # [B, HL, S, D] -> [B, HL, D, S]; stack [x_b^T ; (+-)x_0^T] on the
        # partition axis for b = 1..3 -> [3, HL, 128, S]
        xt = x.transpose(0, 1, 3, 2)  # [B, HL, D, S]
        base = -xt[0] if negate_base else xt[0]  # [HL, D, S]
        stk = np.stack(
            [np.concatenate([xt[b], base], axis=1) for b in (1, 2, 3)], axis=0
        )
        return np.ascontiguousarray(stk).astype(bf16)

    in_maps = []
    for c in range(NCORES):
        hs = slice(HL * c, HL * (c + 1))
        qt = pack_pivot(query[:, hs], negate_base=True)
        kt = pack_pivot(key[:, hs], negate_base=False)
        vc = np.ascontiguousarray(value[:, hs]).astype(bf16)
        in_maps.append({"qt": qt, "kt": kt, "v": vc})

    res = run_bass_kernel_spmd(
        nc, in_maps, core_ids=list(range(NCORES)), trace=TRACE
    )
    LAST_RESULTS = res
    LAST_EXEC_NS = getattr(res, "exec_time_ns", None)

    full = np.empty((B, H, S, D), dtype=np.float32)
    for c in range(NCORES):
        o = np.asarray(res.results[c]["out"])  # [B, HL, D, S]
        full[:, HL * c : HL * (c + 1)] = o.transpose(0, 1, 3, 2)
    return full


# revision 11
# speedup vs baseline: 1.2482x; 1.0083x over previous
"""Trainium2 Bass kernel for nn_Attention_10711648436709.

Math (faithful to reference):
    h = einsum('bhik,bhjk->bhij', Q, K) / sqrt(H)     # scale = sqrt(16) = 4
    w = softmax(h, axis=0)                            # over the BATCH axis (B=4)
    out = einsum('bhij,bhjv->bhiv', w, V)
    (mask is a no-op in the reference)

Sharding: head-parallel across 8 cores (16 heads -> 2 heads/core).
Softmax over batch stays core-local => communication-free.

Per-core layout trick: compute transposed scores S^T[j, i] so that
 - QK:  lhsT = K^T[d, j-block]  rhs = Q^T[d, i-chunk]   (host pre-transposes Q,K)
 - PV:  lhsT = V[j-block, v]    rhs = W[j, i-chunk]     (V in natural layout)
 - output accumulates as out^T[v, i] in PSUM; host transposes back.

Batch-0-pivot softmax: g_b = h_b - h_0 (b=1..3) computed by ONE full-K=128
matmul each: lhsT = [K_b^T ; K_0^T], rhs = [Q_b^T ; -Q_0^T] (host packs).
w_b = e^{g_b/4}/(1 + sum e^{g_b'/4}), w_0 = r = 1/(1 + sum ...).

v2 restructure (vs 361us baseline):
 - process jb in PAIRS with a b-major-pair E layout [128, 3*1024] so every
   softmax elementwise op runs once per pair at double FD (amortizes the
   fixed ~58-224 cycle per-op overheads, DVE TT stays in 2x mode on flat
   step-1 views, one 3-plane W multiply per pair).
 - reciprocal alternates per-pair between the ACT (ln(1+x) via bias, then
   exp(-x); both in the natural_log_exp table set) and an optimized DVE
   Newton chain (seed xor+magic fused into ONE dual-op tensor_scalar, and
   the final (2-dx)*x0 sign-fixed with a fused subtract*(-1) so the chain
   yields +r -- the negated-V side input of the old kernel is gone).
"""

import sys
import os

for p in ("/opt/trn_rl_repo",):
    if p not in sys.path:
        sys.path.insert(0, p)

import numpy as np
import ml_dtypes

B, H, S, D = 4, 16, 2048, 64
NCORES = 8
HL = H // NCORES          # 2 heads per core
NB = S // 128             # 16 j-blocks
NG = 4                    # j-blocks per softmax group ("quad")
NP = NB // NG             # 4 groups per (hl, ic)
NI = S // 512             # 4 i-chunks
GFD = NG * 512            # free dim of one softmax plane group (2048)

# which jb-pairs (of 8 per (hl,ic)) use the ACT reciprocal path; the rest
# use the DVE Newton path.  Balances ACT vs DVE engine load.
ACT_RECIP_PAIRS = (0, 2, 4, 6)
# GPSIMD elementwise offload measured: TT [128,1024] = 2.6us AND its shared
# SBUF port inflates DVE/ACT/PE op times ~20% across the board. Never use.
GPSIMD_T1 = False

TRACE = False
LAST_EXEC_NS = None
LAST_RESULTS = None

_NC = None


def _build_nc():
    import concourse.bass as bass
    import concourse.mybir as mybir
    import concourse.tile as tile

    DT = mybir.dt
    AF = mybir.ActivationFunctionType
    ALU = mybir.AluOpType

    nc = bass.Bass()
    qt = nc.declare_dram_parameter("qt", [3, HL, 128, S], DT.bfloat16, isOutput=False)
    kt = nc.declare_dram_parameter("kt", [3, HL, 128, S], DT.bfloat16, isOutput=False)
    vv = nc.declare_dram_parameter("v", [B, HL, S, D], DT.bfloat16, isOutput=False)
    out = nc.declare_dram_parameter("out", [B, HL, D, S], DT.float32, isOutput=True)

    with tile.TileContext(nc) as tc:
        with (
            tc.tile_pool(name="inputs", bufs=1) as ipool,
            tc.tile_pool(name="work", bufs=3) as wpool,
            tc.tile_pool(name="outsb", bufs=4) as opool,
            tc.tile_pool(name="qkps", bufs=2, space="PSUM") as qkpool,
            tc.tile_pool(name="ops", bufs=1, space="PSUM") as opsum,
        ):
            QT = ipool.tile([128, 3 * HL * S], DT.bfloat16, tag="qt")
            KT = ipool.tile([128, 3 * HL * S], DT.bfloat16, tag="kt")
            VA = ipool.tile([128, B * HL * NB * D], DT.bfloat16, tag="va")
            # load order = consumption order: hl=0's K/Q first (first QK MMs),
            # then its V (first PV is 2 pipeline steps in), then hl=1.
            for hl in range(HL):
                for bb in range(3):
                    off = (bb * HL + hl) * S
                    nc.sync.dma_start(out=KT[:, off : off + S], in_=kt[bb, hl])
                    nc.sync.dma_start(out=QT[:, off : off + S], in_=qt[bb, hl])
                for b in range(B):
                    voff = (b * HL + hl) * NB * D
                    nc.sync.dma_start(
                        out=VA[:, voff : voff + NB * D].rearrange(
                            "p (n d) -> p n d", d=D
                        ),
                        in_=vv[b, hl].rearrange("(n p) d -> p n d", p=128),
                    )

            # 2-stage software pipeline across jb-pairs, flattened over the
            # whole (hl, ic) space: at step t we emit QK+exp for pair t,
            # softmax (DVE/ACT) for pair t-1, and PV matmuls for pair t-2.
            # Every instruction entering an engine FIFO then has its cross-
            # engine deps a full pair-stage old -> no head-of-line stalls,
            # and PE never idles long enough to re-throttle HAM.
            NPAIRS = HL * NI * NP
            pos = {}        # (hl, ic) -> po tiles, created lazily
            state = {}      # pair index -> dict(E, rb, W, jp, hl, ic)

            def pair_coords(t):
                hl, r = divmod(t, NI * NP)
                ic, jp = divmod(r, NP)
                return hl, ic, jp

            def emit_qk_exp(t):
                hl, ic, jp = pair_coords(t)
                E = wpool.tile([128, 3072], DT.bfloat16, tag="E")
                E3d = E.rearrange("p (b n) -> p b n", b=3)
                for h in range(2):
                    jb = 2 * jp + h
                    qk = qkpool.tile([128, 1536], DT.float32, tag="qk")
                    for bb in range(3):
                        off = (bb * HL + hl) * S
                        nc.tensor.matmul(
                            qk[:, bb * 512 : (bb + 1) * 512],
                            lhsT=KT[:, off + jb * 128 : off + jb * 128 + 128],
                            rhs=QT[:, off + ic * 512 : off + ic * 512 + 512],
                            start=True,
                            stop=True,
                        )
                    nc.scalar.activation(
                        E3d[:, :, h * 512 : (h + 1) * 512],
                        qk.rearrange("p (b n) -> p b n", b=3),
                        AF.Exp,
                        scale=0.25,
                    )
                state[t] = {"E": E, "hl": hl, "ic": ic, "jp": jp}

            def emit_softmax(t):
                st = state[t]
                E = st["E"]
                E3d = E.rearrange("p (b n) -> p b n", b=3)
                jp = st["jp"]
                # T1 = E1 + E2, T2 = T1 + E3  (flat [128,1024] bf16, 2x)
                T1 = wpool.tile([128, 1024], DT.bfloat16, tag="T1")
                if GPSIMD_T1:
                    nc.gpsimd.tensor_add(T1, E[:, 0:1024], E[:, 1024:2048])
                else:
                    nc.vector.tensor_add(T1, E[:, 0:1024], E[:, 1024:2048])
                T2 = wpool.tile([128, 1024], DT.bfloat16, tag="T2")
                nc.vector.tensor_add(T2, T1, E[:, 2048:3072])
                rb = wpool.tile([128, 1024], DT.bfloat16, tag="rb")
                if jp in ACT_RECIP_PAIRS:
                    # r = exp(-ln(1 + T2)); Ln and Exp share the
                    # natural_log_exp table set; +1 rides the bias.
                    lnd = wpool.tile([128, 1024], DT.float32, tag="lnd")
                    nc.scalar.activation(lnd, T2, AF.Ln, bias=1.0)
                    nc.scalar.activation(rb, lnd, AF.Exp, scale=-1.0)
                else:
                    # DVE Newton: den = T2 + 1; x0 = bf16 bit seed;
                    # rb = (2 - den*x0) * x0  (sign-fixed to +r).
                    den = wpool.tile([128, 1024], DT.bfloat16, tag="den")
                    nc.vector.tensor_scalar(
                        out=den,
                        in0=T2,
                        scalar1=1.0,
                        scalar2=None,
                        op0=ALU.add,
                    )
                    x0 = wpool.tile([128, 1024], DT.bfloat16, tag="x0")
                    # seed bits = 0x7EF4 + ~den = 0x7EF3 - den, as
                    # mult(-1)+add so both TS ops are arith (the
                    # birverifier rejects mixed bitwise/arith pairs);
                    # int16 values are exact in the fp32 datapath.
                    nc.vector.tensor_scalar(
                        out=x0.bitcast(DT.int16),
                        in0=den.bitcast(DT.int16),
                        scalar1=-1,
                        scalar2=0x7EF3,
                        op0=ALU.mult,
                        op1=ALU.add,
                    )
                    dx = wpool.tile([128, 1024], DT.bfloat16, tag="dx")
                    nc.vector.tensor_mul(dx, den, x0)
                    tp = wpool.tile([128, 1024], DT.bfloat16, tag="tp")
                    # tp = (dx - 2) * (-1) = 2 - dx in one dual-op TS
                    nc.vector.tensor_scalar(
                        out=tp,
                        in0=dx,
                        scalar1=2.0,
                        scalar2=-1.0,
                        op0=ALU.subtract,
                        op1=ALU.mult,
                    )
                    nc.vector.tensor_mul(rb, tp, x0)
                # W_b = E_b * r for b=1..3, one 3-plane broadcast mul
                W = wpool.tile([128, 3072], DT.bfloat16, tag="W")
                nc.vector.tensor_mul(
                    W.rearrange("p (b n) -> p b n", b=3),
                    E3d,
                    rb.unsqueeze(1).broadcast_to([128, 3, 1024]),
                )
                st["rb"] = rb
                st["W"] = W

            def emit_pv(t):
                st = state.pop(t)
                hl, ic, jp = st["hl"], st["ic"], st["jp"]
                rb, W = st["rb"], st["W"]
                if jp == 0:
                    pos[(hl, ic)] = [
                        opsum.tile(
                            [128, 512], DT.float32, tag=f"po{p}", name=f"po{p}"
                        )
                        for p in range(2)
                    ]
                po = pos[(hl, ic)]
                for h in range(2):
                    jb = 2 * jp + h
                    rhss = [
                        rb[:, h * 512 : (h + 1) * 512],
                        W[:, h * 512 : h * 512 + 512],
                        W[:, 1024 + h * 512 : 1024 + h * 512 + 512],
                        W[:, 2048 + h * 512 : 2048 + h * 512 + 512],
                    ]
                    for p in range(2):
                        for half in range(2):
                            b = 2 * p + half
                            voff = (b * HL + hl) * NB * D + jb * D
                            nc.tensor.matmul(
                                po[p][64 * half : 64 * (half + 1), :],
                                lhsT=VA[:, voff : voff + D],
                                rhs=rhss[b],
                                start=(jb == 0),
                                stop=(jb == NB - 1),
                                tile_position=(0, 64 * half),
                            )
                if jp == NP - 1:
                    po = pos.pop((hl, ic))
                    for p in range(2):
                        for half in range(2):
                            b = 2 * p + half
                            osb = opool.tile([D, 512], DT.float32, tag="osb")
                            src = po[p][64 * half : 64 * (half + 1), :]
                            # split the PSUM->SBUF drains between ACT and
                            # DVE to balance engine load
                            if half == 0:
                                nc.scalar.copy(osb, src)
                            else:
                                nc.vector.tensor_copy(osb, src)
                            nc.sync.dma_start(
                                out=out[b, hl, :, ic * 512 : (ic + 1) * 512],
                                in_=osb,
                            )

            # PV (oldest deps) first within each step so PSUM drains and PV
            # matmuls queue ahead of the step's fresh work on each engine.
            for t in range(NPAIRS + 2):
                if t >= 2:
                    emit_pv(t - 2)
                if t < NPAIRS:
                    emit_qk_exp(t)
                if 1 <= t <= NPAIRS:
                    emit_softmax(t - 1)
    return nc


def _patch_bir_waits(bir_json: bytes) -> bytes:
    """This walrus build only accepts 1 sync wait per instruction (2 for
    DMACopy); Tile emits more. Legalize:
      1. merge duplicate-semaphore waits (keep max threshold),
      2. drop waits that are transitively implied (vector-clock replay over
         the straight-line program: in-order completion per engine, FIFO per
         DMA queue, and the knowledge a producer had when it bumped a sem),
      3. split any residual multi-wait onto injected EventSemaphore
         instructions on the same engine right before the instruction.
    Only monotonic sem-inc/sem-ge-imm semaphores participate in (2); barrier
    sems (dec/eq) are left untouched."""
    import json
    from collections import defaultdict

    bir = json.loads(bir_json)

    for fn in bir["functions"]:
        insts = []
        for bb in fn["blocks"]:
            for inst in bb.get("instructions", []):
                insts.append(inst)

        # classify sems: monotonic = all updates are positive sem-inc and
        # all waits are sem-ge-imm
        bad_sems = set()
        for inst in insts:
            si = inst.get("sync_info") or {}
            for u in si.get("on_update") or []:
                if u.get("update_mode") != "sem-inc" or u.get("update_value", 0) <= 0:
                    bad_sems.add(u["id"])
            for w in si.get("on_wait") or []:
                if w.get("wait_mode") != "sem-ge-imm":
                    bad_sems.add(w["id"])

        # proc of an instruction: its engine stream, except DMACopy whose
        # completion (and sem update) is FIFO per DMA queue, keyed by the
        # sem it updates.
        def proc_of(inst):
            if inst.get("opcode") == "DMACopy":
                si = inst.get("sync_info") or {}
                ups = si.get("on_update") or []
                if ups:
                    return ("dma", ups[0]["id"])
            return ("eng", inst.get("engine"))

        sem_val = defaultdict(int)          # current cumulative value per sem
        producers = defaultdict(list)       # sem -> [(value_after, CK dict)]
        know = defaultdict(dict)            # proc -> {sem: guaranteed min}

        def join(dst, src):
            for s, v in src.items():
                if dst.get(s, 0) < v:
                    dst[s] = v

        out_blocks = {id(bb): [] for bb in fn["blocks"]}
        inj = 0
        for bb in fn["blocks"]:
            new_list = []
            for inst in bb.get("instructions", []):
                p = proc_of(inst)
                eng_p = ("eng", inst.get("engine"))
                # waits on a DMACopy are enforced by the DGE queue (FIFO per
                # queue), not the issuing engine — track knowledge per queue
                kp = p if p[0] == "dma" else eng_p
                si = inst.get("sync_info") or {}
                waits = si.get("on_wait") or []
                # merge duplicate sems
                merged = {}
                for w in waits:
                    k = w["id"]
                    if k not in merged or w.get("wait_value", 0) > merged[k].get(
                        "wait_value", 0
                    ):
                        merged[k] = w
                waits = list(merged.values())
                kept = []
                for w in waits:
                    s, v = w["id"], w.get("wait_value", 0)
                    if s in bad_sems:
                        kept.append(w)
                        continue
                    if know[kp].get(s, 0) >= v:
                        continue  # redundant
                    kept.append(w)
                    know[kp][s] = max(know[kp].get(s, 0), v)
                    # transitive knowledge from the producer that reached v
                    for val_after, ck in producers[s]:
                        if val_after >= v:
                            join(know[kp], ck)
                            break
                # split if too many waits remain
                budget = 1
                while len(kept) > budget:
                    w = kept.pop(0)
                    inj += 1
                    new_list.append(
                        {
                            "debug": inst.get("debug", 0),
                            "engine": inst.get("engine"),
                            "ins": [],
                            "name": f"WS-{inj}-{inst.get('name')}",
                            "opcode": "EventSemaphore",
                            "outs": [],
                            "sync_info": {"on_update": [], "on_wait": [w]},
                        }
                    )
                si["on_wait"] = kept
                inst["sync_info"] = si
                new_list.append(inst)
                # apply this instruction's updates for downstream knowledge
                ups = si.get("on_update") or []
                ck = None
                for u in ups:
                    s = u["id"]
                    if s in bad_sems:
                        continue
                    sem_val[s] += u.get("update_value", 0)
                    if ck is None:
                        # completion knowledge: what this proc knew here
                        # (for DMA: queue knowledge + engine state at issue)
                        ck = dict(know[kp])
                        if p[0] == "dma":
                            join(ck, know[eng_p])
                    ck[s] = sem_val[s]
                    producers[s].append((sem_val[s], ck))
                # a proc knows its own sems' values after completion
                if p[0] == "eng":
                    for u in ups:
                        if u["id"] not in bad_sems:
                            know[eng_p][u["id"]] = sem_val[u["id"]]
            out_blocks[id(bb)] = new_list
        for bb in fn["blocks"]:
            bb["instructions"] = out_blocks[id(bb)]
    return json.dumps(bir).encode()


_PATCHED = False


def _install_bir_patch():
    global _PATCHED
    if _PATCHED:
        return
    import concourse.bass2jax as bass2jax
    from concourse import bass_utils as _bu

    orig = _bu.compile_bir_kernel

    def patched(bir_json, tmpdir, neff_name="file.neff"):
        return orig(_patch_bir_waits(bir_json), tmpdir, neff_name)

    bass2jax.compile_bir_kernel = patched
    # keep profile artifacts local — no bucket in this environment
    _bu.upload_artifacts = lambda tmpdir: str(tmpdir)
    _PATCHED = True


def _install_ntff_shim():
    """run_bass_kernel_spmd(trace=True) under axon needs
    antenv.axon_hooks.get_axon_ntff_profile_hook; the module isn't staged in
    this image, but libaxon_pjrt.so exposes the profile C ABI — recreate the
    shim (same recipe as trn_agent_boot)."""
    import sys as _sys

    if "antenv.axon_hooks" in _sys.modules:
        return
    import contextlib
    import ctypes
    import types

    import antenv  # noqa: F401

    so_path = "/opt/axon/libaxon_pjrt.so"
    hook = None
    try:
        lib = ctypes.CDLL(so_path)
        if hasattr(lib, "axon_start_nrt_profile"):
            lib.axon_start_nrt_profile.argtypes = [
                ctypes.POINTER(ctypes.c_int64),
                ctypes.c_size_t,
            ]
            lib.axon_start_nrt_profile.restype = ctypes.c_int64
            lib.axon_stop_nrt_profile.argtypes = [ctypes.c_char_p]
            lib.axon_stop_nrt_profile.restype = ctypes.c_int64

            @contextlib.contextmanager
            def hook(output_dir, device_ids):
                import jax

                jax.devices()
                if device_ids:
                    ids = (ctypes.c_int64 * len(device_ids))(*device_ids)
                    rc = lib.axon_start_nrt_profile(ids, len(device_ids))
                else:
                    rc = lib.axon_start_nrt_profile(None, 0)
                if rc != 0:
                    raise RuntimeError(f"axon_start_nrt_profile rc={rc}")
                try:
                    yield
                finally:
                    n = lib.axon_stop_nrt_profile(str(output_dir).encode())
                    print(
                        f"ntff profile: {n} file(s) -> {output_dir}",
                        file=_sys.stderr,
                    )
    except OSError:
        pass

    mod = types.ModuleType("antenv.axon_hooks")
    mod.get_axon_ntff_profile_hook = lambda: hook
    mod.set_axon_ntff_profile_hook = lambda h: None
    _sys.modules["antenv.axon_hooks"] = mod
    import antenv as _ae

    _ae.axon_hooks = mod


def kernel(query, key, value, mask=None):
    global _NC, LAST_EXEC_NS, LAST_RESULTS
    from concourse.bass_utils import run_bass_kernel_spmd

    _install_bir_patch()
    if TRACE:
        _install_ntff_shim()

    query = np.asarray(query, dtype=np.float32)
    key = np.asarray(key, dtype=np.float32)
    value = np.asarray(value, dtype=np.float32)

    if _NC is None:
        _NC = _build_nc()
    nc = _NC

    bf16 = ml_dtypes.bfloat16

    def pack_pivot(x, negate_base):
        # [B, HL, S, D] -> [B, HL, D, S]; stack [x_b^T ; (+-)x_0^T] on the
        # partition axis for b = 1..3 -> [3, HL, 128, S]
        xt = x.transpose(0, 1, 3, 2)  # [B, HL, D, S]
        base = -xt[0] if negate_base else xt[0]  # [HL, D, S]
        stk = np.stack(
            [np.concatenate([xt[b], base], axis=1) for b in (1, 2, 3)], axis=0
        )
        return np.ascontiguousarray(stk).astype(bf16)

    in_maps = []
    for c in range(NCORES):
        hs = slice(HL * c, HL * (c + 1))
        qt = pack_pivot(query[:, hs], negate_base=True)
        kt = pack_pivot(key[:, hs], negate_base=False)
        vc = np.ascontiguousarray(value[:, hs]).astype(bf16)
        in_maps.append({"qt": qt, "kt": kt, "v": vc})

    res = run_bass_kernel_spmd(
        nc, in_maps, core_ids=list(range(NCORES)), trace=TRACE
    )
    LAST_RESULTS = res
    LAST_EXEC_NS = getattr(res, "exec_time_ns", None)

    full = np.empty((B, H, S, D), dtype=np.float32)
    for c in range(NCORES):
        o = np.asarray(res.results[c]["out"])  # [B, HL, D, S]
        full[:, HL * c : HL * (c + 1)] = o.transpose(0, 1, 3, 2)
    return full


# revision 16
# speedup vs baseline: 1.2648x; 1.0133x over previous
"""Trainium2 Bass kernel for nn_Attention_10711648436709.

Math (faithful to reference):
    h = einsum('bhik,bhjk->bhij', Q, K) / sqrt(H)     # scale = sqrt(16) = 4
    w = softmax(h, axis=0)                            # over the BATCH axis (B=4)
    out = einsum('bhij,bhjv->bhiv', w, V)
    (mask is a no-op in the reference)

Sharding: head-parallel across 8 cores (16 heads -> 2 heads/core).
Softmax over batch stays core-local => communication-free.

Per-core layout trick: compute transposed scores S^T[j, i] so that
 - QK:  lhsT = K^T[d, j-block]  rhs = Q^T[d, i-chunk]   (host pre-transposes Q,K)
 - PV:  lhsT = V[j-block, v]    rhs = W[j, i-chunk]     (V in natural layout)
 - output accumulates as out^T[v, i] in PSUM; host transposes back.

Batch-0-pivot softmax: g_b = h_b - h_0 (b=1..3) computed by ONE full-K=128
matmul each: lhsT = [K_b^T ; K_0^T], rhs = [Q_b^T ; -Q_0^T] (host packs).
w_b = e^{g_b/4}/(1 + sum e^{g_b'/4}), w_0 = r = 1/(1 + sum ...).

v2 restructure (vs 361us baseline):
 - process jb in PAIRS with a b-major-pair E layout [128, 3*1024] so every
   softmax elementwise op runs once per pair at double FD (amortizes the
   fixed ~58-224 cycle per-op overheads, DVE TT stays in 2x mode on flat
   step-1 views, one 3-plane W multiply per pair).
 - reciprocal alternates per-pair between the ACT (ln(1+x) via bias, then
   exp(-x); both in the natural_log_exp table set) and an optimized DVE
   Newton chain (seed xor+magic fused into ONE dual-op tensor_scalar, and
   the final (2-dx)*x0 sign-fixed with a fused subtract*(-1) so the chain
   yields +r -- the negated-V side input of the old kernel is gone).
"""

import sys
import os

for p in ("/opt/trn_rl_repo",):
    if p not in sys.path:
        sys.path.insert(0, p)

import numpy as np
import ml_dtypes

B, H, S, D = 4, 16, 2048, 64
NCORES = 8
HL = H // NCORES          # 2 heads per core
NB = S // 128             # 16 j-blocks
NG = 4                    # j-blocks per softmax group ("quad")
NP = NB // NG             # 4 groups per (hl, ic)
NI = S // 512             # 4 i-chunks
GFD = NG * 512            # free dim of one softmax plane group (2048)

# which jb-pairs (of 8 per (hl,ic)) use the ACT reciprocal path; the rest
# use the DVE Newton path.  Balances ACT vs DVE engine load.
ACT_RECIP_GROUPS = (0, 2)   # of the NP=4 groups per (hl, ic): half on ACT
# GPSIMD elementwise offload measured: TT [128,1024] = 2.6us AND its shared
# SBUF port inflates DVE/ACT/PE op times ~20% across the board. Never use.
GPSIMD_T1 = False

TRACE = False
LAST_EXEC_NS = None
LAST_RESULTS = None

_NC = None


def _build_nc():
    import concourse.bass as bass
    import concourse.mybir as mybir
    import concourse.tile as tile

    DT = mybir.dt
    AF = mybir.ActivationFunctionType
    ALU = mybir.AluOpType

    nc = bass.Bass()
    qt = nc.declare_dram_parameter("qt", [3, HL, 128, S], DT.bfloat16, isOutput=False)
    kt = nc.declare_dram_parameter("kt", [3, HL, 128, S], DT.bfloat16, isOutput=False)
    vv = nc.declare_dram_parameter("v", [B, HL, S, D], DT.bfloat16, isOutput=False)
    out = nc.declare_dram_parameter("out", [B, HL, D, S], DT.float32, isOutput=True)

    with tile.TileContext(nc) as tc:
        with (
            tc.tile_pool(name="inputs", bufs=1) as ipool,
            tc.tile_pool(name="wbig", bufs=2) as wbig,
            tc.tile_pool(name="wchain", bufs=2) as wpool,
            tc.tile_pool(name="outsb", bufs=4) as opool,
            tc.tile_pool(name="qkps", bufs=2, space="PSUM") as qkpool,
            tc.tile_pool(name="ops", bufs=1, space="PSUM") as opsum,
        ):
            QT = ipool.tile([128, 3 * HL * S], DT.bfloat16, tag="qt")
            KT = ipool.tile([128, 3 * HL * S], DT.bfloat16, tag="kt")
            VA = ipool.tile([128, B * HL * NB * D], DT.bfloat16, tag="va")
            # load order = consumption order: the first group's K columns and
            # ic0's Q columns for hl=0 land first (small transfers) so the
            # first QK matmuls can start ~4us in; remainders + V follow.
            KC = NG * 128  # KT columns consumed by the first jb-group
            for bb in range(3):
                off = bb * HL * S
                nc.sync.dma_start(
                    out=KT[:, off : off + KC], in_=kt[bb, 0, :, 0:KC]
                )
                nc.sync.dma_start(
                    out=QT[:, off : off + 512], in_=qt[bb, 0, :, 0:512]
                )
            for hl in range(HL):
                for bb in range(3):
                    off = (bb * HL + hl) * S
                    if hl == 0:
                        nc.sync.dma_start(
                            out=KT[:, off + KC : off + S],
                            in_=kt[bb, hl, :, KC:S],
                        )
                        nc.sync.dma_start(
                            out=QT[:, off + 512 : off + S],
                            in_=qt[bb, hl, :, 512:S],
                        )
                    else:
                        nc.sync.dma_start(
                            out=KT[:, off : off + S], in_=kt[bb, hl]
                        )
                        nc.sync.dma_start(
                            out=QT[:, off : off + S], in_=qt[bb, hl]
                        )
                for b in range(B):
                    voff = (b * HL + hl) * NB * D
                    nc.sync.dma_start(
                        out=VA[:, voff : voff + NB * D].rearrange(
                            "p (n d) -> p n d", d=D
                        ),
                        in_=vv[b, hl].rearrange("(n p) d -> p n d", p=128),
                    )

            # 2-stage software pipeline across jb-groups (NG=4 j-blocks per
            # group), flattened over the whole (hl, ic) space: at step t we
            # emit QK+exp for group t, softmax (DVE/ACT) for group t-1, and
            # PV matmuls for group t-2.  Every instruction entering an
            # engine FIFO then has its cross-engine deps a full stage old ->
            # no head-of-line stalls, and PE never idles long enough to
            # re-throttle HAM.  Grouping at NG=4 runs each softmax op once
            # per group at 4x FD, amortizing the fixed per-op overheads.
            NSTEPS = HL * NI * NP
            pos = {}        # (hl, ic) -> po tiles, created lazily
            state = {}      # step index -> dict(E, rb, W, jp, hl, ic)

            def step_coords(t):
                hl, r = divmod(t, NI * NP)
                ic, jp = divmod(r, NP)
                return hl, ic, jp

            def emit_qk_exp(t):
                hl, ic, jp = step_coords(t)
                # E layout: b-major-group [128, 3*GFD]: plane b at
                # [b*GFD : (b+1)*GFD]; within a plane, jb-subblock h at
                # [h*512 : (h+1)*512]
                E = wbig.tile([128, 3 * GFD], DT.bfloat16, tag="E")
                E3d = E.rearrange("p (b n) -> p b n", b=3)
                for h in range(NG):
                    jb = NG * jp + h
                    qk = qkpool.tile([128, 1536], DT.float32, tag="qk")
                    for bb in range(3):
                        off = (bb * HL + hl) * S
                        nc.tensor.matmul(
                            qk[:, bb * 512 : (bb + 1) * 512],
                            lhsT=KT[:, off + jb * 128 : off + jb * 128 + 128],
                            rhs=QT[:, off + ic * 512 : off + ic * 512 + 512],
                            start=True,
                            stop=True,
                        )
                    nc.scalar.activation(
                        E3d[:, :, h * 512 : (h + 1) * 512],
                        qk.rearrange("p (b n) -> p b n", b=3),
                        AF.Exp,
                        scale=0.25,
                    )
                state[t] = {"E": E, "hl": hl, "ic": ic, "jp": jp}

            def emit_softmax(t):
                st = state[t]
                E = st["E"]
                E3d = E.rearrange("p (b n) -> p b n", b=3)
                jp = st["jp"]
                # T1 = E1 + E2, T2 = T1 + E3  (flat [128,GFD] bf16, 2x)
                T1 = wpool.tile([128, GFD], DT.bfloat16, tag="T1")
                nc.vector.tensor_add(T1, E[:, 0:GFD], E[:, GFD : 2 * GFD])
                T2 = wpool.tile([128, GFD], DT.bfloat16, tag="T2")
                nc.vector.tensor_add(T2, T1, E[:, 2 * GFD : 3 * GFD])
                rb = wpool.tile([128, GFD], DT.bfloat16, tag="rb")
                if jp in ACT_RECIP_GROUPS:
                    # r = exp(-ln(1 + T2)); Ln and Exp share the
                    # natural_log_exp table set; +1 rides the bias.
                    lnd = wpool.tile([128, GFD], DT.float32, tag="lnd")
                    nc.scalar.activation(lnd, T2, AF.Ln, bias=1.0)
                    nc.scalar.activation(rb, lnd, AF.Exp, scale=-1.0)
                else:
                    # DVE Newton: den = T2 + 1; x0 = bf16 bit seed;
                    # rb = (2 - den*x0) * x0  (sign-fixed to +r).
                    den = wpool.tile([128, GFD], DT.bfloat16, tag="den")
                    nc.vector.tensor_scalar(
                        out=den,
                        in0=T2,
                        scalar1=1.0,
                        scalar2=None,
                        op0=ALU.add,
                    )
                    x0 = wpool.tile([128, GFD], DT.bfloat16, tag="x0")
                    # seed bits = 0x7EF4 + ~den = 0x7EF3 - den, as
                    # mult(-1)+add so both TS ops are arith (the
                    # birverifier rejects mixed bitwise/arith pairs);
                    # int16 values are exact in the fp32 datapath.
                    nc.vector.tensor_scalar(
                        out=x0.bitcast(DT.int16),
                        in0=den.bitcast(DT.int16),
                        scalar1=-1,
                        scalar2=0x7EF3,
                        op0=ALU.mult,
                        op1=ALU.add,
                    )
                    dx = wpool.tile([128, GFD], DT.bfloat16, tag="dx")
                    nc.vector.tensor_mul(dx, den, x0)
                    tp = wpool.tile([128, GFD], DT.bfloat16, tag="tp")
                    # tp = (dx - 2) * (-1) = 2 - dx in one dual-op TS
                    nc.vector.tensor_scalar(
                        out=tp,
                        in0=dx,
                        scalar1=2.0,
                        scalar2=-1.0,
                        op0=ALU.subtract,
                        op1=ALU.mult,
                    )
                    nc.vector.tensor_mul(rb, tp, x0)
                # W_b = E_b * r for b=1..3, one 3-plane broadcast mul
                W = wbig.tile([128, 3 * GFD], DT.bfloat16, tag="W")
                nc.vector.tensor_mul(
                    W.rearrange("p (b n) -> p b n", b=3),
                    E3d,
                    rb.unsqueeze(1).broadcast_to([128, 3, GFD]),
                )
                st["rb"] = rb
                st["W"] = W

            def emit_pv(t):
                st = state.pop(t)
                hl, ic, jp = st["hl"], st["ic"], st["jp"]
                rb, W = st["rb"], st["W"]
                if jp == 0:
                    pos[(hl, ic)] = [
                        opsum.tile(
                            [128, 512], DT.float32, tag=f"po{p}", name=f"po{p}"
                        )
                        for p in range(2)
                    ]
                po = pos[(hl, ic)]
                for h in range(NG):
                    jb = NG * jp + h
                    rhss = [
                        rb[:, h * 512 : (h + 1) * 512],
                        W[:, h * 512 : h * 512 + 512],
                        W[:, GFD + h * 512 : GFD + h * 512 + 512],
                        W[:, 2 * GFD + h * 512 : 2 * GFD + h * 512 + 512],
                    ]
                    for p in range(2):
                        for half in range(2):
                            b = 2 * p + half
                            voff = (b * HL + hl) * NB * D + jb * D
                            nc.tensor.matmul(
                                po[p][64 * half : 64 * (half + 1), :],
                                lhsT=VA[:, voff : voff + D],
                                rhs=rhss[b],
                                start=(jb == 0),
                                stop=(jb == NB - 1),
                                tile_position=(0, 64 * half),
                            )
                if jp == NP - 1:
                    po = pos.pop((hl, ic))
                    for p in range(2):
                        for half in range(2):
                            b = 2 * p + half
                            osb = opool.tile([D, 512], DT.float32, tag="osb")
                            src = po[p][64 * half : 64 * (half + 1), :]
                            # split the PSUM->SBUF drains between ACT and
                            # DVE to balance engine load
                            if half == 0:
                                nc.scalar.copy(osb, src)
                            else:
                                nc.vector.tensor_copy(osb, src)
                            nc.sync.dma_start(
                                out=out[b, hl, :, ic * 512 : (ic + 1) * 512],
                                in_=osb,
                            )

            # PV (oldest deps) first within each step so PSUM drains and PV
            # matmuls queue ahead of the step's fresh work on each engine.
            for t in range(NSTEPS + 2):
                if t >= 2:
                    emit_pv(t - 2)
                if t < NSTEPS:
                    emit_qk_exp(t)
                if 1 <= t <= NSTEPS:
                    emit_softmax(t - 1)
    return nc


def _patch_bir_waits(bir_json: bytes) -> bytes:
    """This walrus build only accepts 1 sync wait per instruction (2 for
    DMACopy); Tile emits more. Legalize:
      1. merge duplicate-semaphore waits (keep max threshold),
      2. drop waits that are transitively implied (vector-clock replay over
         the straight-line program: in-order completion per engine, FIFO per
         DMA queue, and the knowledge a producer had when it bumped a sem),
      3. split any residual multi-wait onto injected EventSemaphore
         instructions on the same engine right before the instruction.
    Only monotonic sem-inc/sem-ge-imm semaphores participate in (2); barrier
    sems (dec/eq) are left untouched."""
    import json
    from collections import defaultdict

    bir = json.loads(bir_json)

    for fn in bir["functions"]:
        insts = []
        for bb in fn["blocks"]:
            for inst in bb.get("instructions", []):
                insts.append(inst)

        # classify sems: monotonic = all updates are positive sem-inc and
        # all waits are sem-ge-imm
        bad_sems = set()
        for inst in insts:
            si = inst.get("sync_info") or {}
            for u in si.get("on_update") or []:
                if u.get("update_mode") != "sem-inc" or u.get("update_value", 0) <= 0:
                    bad_sems.add(u["id"])
            for w in si.get("on_wait") or []:
                if w.get("wait_mode") != "sem-ge-imm":
                    bad_sems.add(w["id"])

        # proc of an instruction: its engine stream, except DMACopy whose
        # completion (and sem update) is FIFO per DMA queue, keyed by the
        # sem it updates.
        def proc_of(inst):
            if inst.get("opcode") == "DMACopy":
                si = inst.get("sync_info") or {}
                ups = si.get("on_update") or []
                if ups:
                    return ("dma", ups[0]["id"])
            return ("eng", inst.get("engine"))

        sem_val = defaultdict(int)          # current cumulative value per sem
        producers = defaultdict(list)       # sem -> [(value_after, CK dict)]
        know = defaultdict(dict)            # proc -> {sem: guaranteed min}

        def join(dst, src):
            for s, v in src.items():
                if dst.get(s, 0) < v:
                    dst[s] = v

        out_blocks = {id(bb): [] for bb in fn["blocks"]}
        inj = 0
        for bb in fn["blocks"]:
            new_list = []
            for inst in bb.get("instructions", []):
                p = proc_of(inst)
                eng_p = ("eng", inst.get("engine"))
                # waits on a DMACopy are enforced by the DGE queue (FIFO per
                # queue), not the issuing engine — track knowledge per queue
                kp = p if p[0] == "dma" else eng_p
                si = inst.get("sync_info") or {}
                waits = si.get("on_wait") or []
                # merge duplicate sems
                merged = {}
                for w in waits:
                    k = w["id"]
                    if k not in merged or w.get("wait_value", 0) > merged[k].get(
                        "wait_value", 0
                    ):
                        merged[k] = w
                waits = list(merged.values())
                kept = []
                for w in waits:
                    s, v = w["id"], w.get("wait_value", 0)
                    if s in bad_sems:
                        kept.append(w)
                        continue
                    if know[kp].get(s, 0) >= v:
                        continue  # redundant
                    kept.append(w)
                    know[kp][s] = max(know[kp].get(s, 0), v)
                    # transitive knowledge from the producer that reached v
                    for val_after, ck in producers[s]:
                        if val_after >= v:
                            join(know[kp], ck)
                            break
                # split if too many waits remain
                budget = 1
                while len(kept) > budget:
                    w = kept.pop(0)
                    inj += 1
                    new_list.append(
                        {
                            "debug": inst.get("debug", 0),
                            "engine": inst.get("engine"),
                            "ins": [],
                            "name": f"WS-{inj}-{inst.get('name')}",
                            "opcode": "EventSemaphore",
                            "outs": [],
                            "sync_info": {"on_update": [], "on_wait": [w]},
                        }
                    )
                si["on_wait"] = kept
                inst["sync_info"] = si
                new_list.append(inst)
                # apply this instruction's updates for downstream knowledge
                ups = si.get("on_update") or []
                ck = None
                for u in ups:
                    s = u["id"]
                    if s in bad_sems:
                        continue
                    sem_val[s] += u.get("update_value", 0)
                    if ck is None:
                        # completion knowledge: what this proc knew here
                        # (for DMA: queue knowledge + engine state at issue)
                        ck = dict(know[kp])
                        if p[0] == "dma":
                            join(ck, know[eng_p])
                    ck[s] = sem_val[s]
                    producers[s].append((sem_val[s], ck))
                # a proc knows its own sems' values after completion
                if p[0] == "eng":
                    for u in ups:
                        if u["id"] not in bad_sems:
                            know[eng_p][u["id"]] = sem_val[u["id"]]
            out_blocks[id(bb)] = new_list
        for bb in fn["blocks"]:
            bb["instructions"] = out_blocks[id(bb)]
    return json.dumps(bir).encode()


_PATCHED = False


def _install_bir_patch():
    global _PATCHED
    if _PATCHED:
        return
    import concourse.bass2jax as bass2jax
    from concourse import bass_utils as _bu

    orig = _bu.compile_bir_kernel

    def patched(bir_json, tmpdir, neff_name="file.neff"):
        return orig(_patch_bir_waits(bir_json), tmpdir, neff_name)

    bass2jax.compile_bir_kernel = patched
    # keep profile artifacts local — no bucket in this environment
    _bu.upload_artifacts = lambda tmpdir: str(tmpdir)
    _PATCHED = True


def _install_ntff_shim():
    """run_bass_kernel_spmd(trace=True) under axon needs
    antenv.axon_hooks.get_axon_ntff_profile_hook; the module isn't staged in
    this image, but libaxon_pjrt.so exposes the profile C ABI — recreate the
    shim (same recipe as trn_agent_boot)."""
    import sys as _sys

    if "antenv.axon_hooks" in _sys.modules:
        return
    import contextlib
    import ctypes
    import types

    import antenv  # noqa: F401

    so_path = "/opt/axon/libaxon_pjrt.so"
    hook = None
    try:
        lib = ctypes.CDLL(so_path)
        if hasattr(lib, "axon_start_nrt_profile"):
            lib.axon_start_nrt_profile.argtypes = [
                ctypes.POINTER(ctypes.c_int64),
                ctypes.c_size_t,
            ]
            lib.axon_start_nrt_profile.restype = ctypes.c_int64
            lib.axon_stop_nrt_profile.argtypes = [ctypes.c_char_p]
            lib.axon_stop_nrt_profile.restype = ctypes.c_int64

            @contextlib.contextmanager
            def hook(output_dir, device_ids):
                import jax

                jax.devices()
                if device_ids:
                    ids = (ctypes.c_int64 * len(device_ids))(*device_ids)
                    rc = lib.axon_start_nrt_profile(ids, len(device_ids))
                else:
                    rc = lib.axon_start_nrt_profile(None, 0)
                if rc != 0:
                    raise RuntimeError(f"axon_start_nrt_profile rc={rc}")
                try:
                    yield
                finally:
                    n = lib.axon_stop_nrt_profile(str(output_dir).encode())
                    print(
                        f"ntff profile: {n} file(s) -> {output_dir}",
                        file=_sys.stderr,
                    )
    except OSError:
        pass

    mod = types.ModuleType("antenv.axon_hooks")
    mod.get_axon_ntff_profile_hook = lambda: hook
    mod.set_axon_ntff_profile_hook = lambda h: None
    _sys.modules["antenv.axon_hooks"] = mod
    import antenv as _ae

    _ae.axon_hooks = mod


def kernel(query, key, value, mask=None):
    global _NC, LAST_EXEC_NS, LAST_RESULTS
    from concourse.bass_utils import run_bass_kernel_spmd

    _install_bir_patch()
    if TRACE:
        _install_ntff_shim()

    query = np.asarray(query, dtype=np.float32)
    key = np.asarray(key, dtype=np.float32)
    value = np.asarray(value, dtype=np.float32)

    if _NC is None:
        _NC = _build_nc()
    nc = _NC

    bf16 = ml_dtypes.bfloat16

    def pack_pivot(x, negate_base):
        # [B, HL, S, D] -> [B, HL, D, S]; stack [x_b^T ; (+-)x_0^T] on the
        # partition axis for b = 1..3 -> [3, HL, 128, S]
        xt = x.transpose(0, 1, 3, 2)  # [B, HL, D, S]
        base = -xt[0] if negate_base else xt[0]  # [HL, D, S]
        stk = np.stack(
            [np.concatenate([xt[b], base], axis=1) for b in (1, 2, 3)], axis=0
        )
        return np.ascontiguousarray(stk).astype(bf16)

    in_maps = []
    for c in range(NCORES):
        hs = slice(HL * c, HL * (c + 1))
        qt = pack_pivot(query[:, hs], negate_base=True)
        kt = pack_pivot(key[:, hs], negate_base=False)
        vc = np.ascontiguousarray(value[:, hs]).astype(bf16)
        in_maps.append({"qt": qt, "kt": kt, "v": vc})

    res = run_bass_kernel_spmd(
        nc, in_maps, core_ids=list(range(NCORES)), trace=TRACE
    )
    LAST_RESULTS = res
    LAST_EXEC_NS = getattr(res, "exec_time_ns", None)

    full = np.empty((B, H, S, D), dtype=np.float32)
    for c in range(NCORES):
        o = np.asarray(res.results[c]["out"])  # [B, HL, D, S]
        full[:, HL * c : HL * (c + 1)] = o.transpose(0, 1, 3, 2)
    return full
